# revision 15
# baseline (speedup 1.0000x reference)
"""CTC loss (keras ctc_batch_cost semantics) on 8 Trainium2 NeuronCores.

Strategy (pure data parallelism, batch sharded 128 samples/core):
  - DP runs in probability space with periodic per-sample rescaling:
        P[t,s] = y_ext[t,s] * (P[t-1,s] + P[t-1,s-1] + allow_skip*P[t-1,s-2])
    Samples ride the 128 SBUF partitions; the S=129 lattice states live in
    the free dimension of [128, S]-shaped DVE ops.
  - The per-(sample,t) emission gather y_pred[b,t,ext(b,s)] is done with
    per-sample one-hot matmuls on the PE array:
        PE transpose  y[b]  [T,C] -> [C,T]   (128x128 blocks)
        G[b] = W[b].T @ yT[b]   with W[b] [C,128] = packed one-hots:
            cols 0..63  : onehot(lab[l])                (odd-state emissions)
            cols 64..127: onehot(lab[l]) * allow_skip   (skip-masked copy)
    Per time step a second PE transpose turns G[:, t-slice, b] into a
    [128b, 128m] tile the DVE consumes directly from PSUM.
  - Blank emissions (even lattice states) multiply by a per-partition scalar
    plane ybe[b,t] = y_pred[b,t,C-1]+EPS (ScalarE activation with scale-AP).
  - Loss = -(log(P[2L] + P[2L-1]) + sum of rescale logs).
"""

import numpy as np

B, T, C, L = 1024, 512, 256, 64
S = 2 * L + 1  # 129
NCORES = 8
BL = B // NCORES  # 128 samples per core
EPS = 1e-7
RBLK = 8  # rescale period (time steps)
# Static per-state exponential tilt P~[s] = P[s]*exp(-G_TILT*s). Flattens the
# lattice's s-profile so all answer-relevant states fit f32 range; folded into
# the sh1 scalar, the host-built W2/end-mask, and the logacc initialization.
G_TILT = 1.75
OFFS = 30.0  # rescale offset: row max is normalized to e^OFFS, not 1

_prog = None  # cached compiled Bass program
_last_results = None


def _build_program():
    from contextlib import ExitStack

    import concourse.bacc as bacc
    import concourse.bass as bass
    import concourse.mybir as mybir
    import concourse.tile as tile

    F32 = mybir.dt.float32
    BF16 = mybir.dt.bfloat16
    OP = mybir.AluOpType
    AF = mybir.ActivationFunctionType
    AX = mybir.AxisListType
    PSUM = bass.MemorySpace.PSUM

    TCH = 128            # time-chunk length
    NCH = T // TCH       # 4 chunks
    NQ = BL // 4         # sample quads per chunk
    E1 = float(np.exp(-G_TILT))
    OFFE = float(np.exp(OFFS))

    nc = bacc.Bacc("TRN2", target_bir_lowering=False, debug=False)

    yp_d = nc.dram_tensor("yp", [BL, T, C], F32, kind="ExternalInput").ap()
    wg_d = nc.dram_tensor("wg", [BL, 2, 128, 128], BF16, kind="ExternalInput").ap()
    ybe_d = nc.dram_tensor("ybe", [BL, T], F32, kind="ExternalInput").ap()
    em_d = nc.dram_tensor("em", [BL, S], F32, kind="ExternalInput").ap()
    idf_d = nc.dram_tensor("idf", [128, 128], F32, kind="ExternalInput").ap()
    pend_d = nc.dram_tensor("pend", [BL, 1], F32, kind="ExternalOutput").ap()
    mxh_d = nc.dram_tensor("mxh", [BL, T // RBLK], F32, kind="ExternalOutput").ap()

    with tile.TileContext(nc) as tc, ExitStack() as ctx:
        # ---- persistent SBUF state (one pool, unique tags) ----
        per = ctx.enter_context(tc.tile_pool(name="per", bufs=1))
        ybe_sb = per.tile([128, T], F32, tag="ybe", name="ybe_sb")
        em_sb = per.tile([128, S], F32, tag="em", name="em_sb")
        idf = per.tile([128, 128], F32, tag="idf", name="idf_sb")
        pa = per.tile([128, 131], F32, tag="pa", name="pa")
        pb = per.tile([128, 131], F32, tag="pb", name="pb")
        mxh = per.tile([128, T // RBLK], F32, tag="mxh", name="mxh")

        nc.sync.dma_start(ybe_sb[:], ybe_d)
        nc.sync.dma_start(em_sb[:], em_d)
        nc.sync.dma_start(idf[:], idf_d)
        nc.vector.memset(pa[:], 0.0)
        nc.vector.memset(pb[:], 0.0)

        # ---- pools ----
        yin = ctx.enter_context(tc.tile_pool(name="yin", bufs=6))
        ytp = ctx.enter_context(tc.tile_pool(name="ytp", bufs=4))
        wpl = ctx.enter_context(tc.tile_pool(name="wpl", bufs=6))
        gcp = ctx.enter_context(tc.tile_pool(name="gcp", bufs=2))
        apl = ctx.enter_context(tc.tile_pool(name="apl", bufs=2))
        vpl = ctx.enter_context(tc.tile_pool(name="vpl", bufs=2))
        spl = ctx.enter_context(tc.tile_pool(name="spl", bufs=6))
        tpp = ctx.enter_context(tc.tile_pool(name="tpp", space=PSUM, bufs=4))
        gpp = ctx.enter_context(tc.tile_pool(name="gpp", space=PSUM, bufs=2))
        yyp = ctx.enter_context(tc.tile_pool(name="yyp", space=PSUM, bufs=2))

        gc3 = {}  # chunk -> [128m, TCH, 128b] SBUF view (f32)

        def gather_chunk(k):
            g = gcp.tile([128, TCH * 128], F32, tag="gc")
            g3 = g[:].rearrange("p (t b) -> p t b", b=128)
            gc3[k] = g3
            for q in range(NQ):
                tp0 = tpp.tile([128, 512], F32, tag="tp")
                tp1 = tpp.tile([128, 512], F32, tag="tp")
                yt0 = ytp.tile([128, 512], BF16, tag="yt")
                yt1 = ytp.tile([128, 512], BF16, tag="yt")
                ws = []
                for si in range(4):
                    smp = q * 4 + si
                    y = yin.tile([128, 256], F32, tag="yin")
                    nc.sync.dma_start(y[:], yp_d[smp, k * TCH:(k + 1) * TCH, :])
                    nc.tensor.transpose(tp0[:, si * 128:(si + 1) * 128], y[:, 0:128], idf[:])
                    nc.tensor.transpose(tp1[:, si * 128:(si + 1) * 128], y[:, 128:256], idf[:])
                    w = wpl.tile([128, 256], BF16, tag="w")
                    w3 = w[:].rearrange("c (ck m) -> c ck m", ck=2)
                    nc.sync.dma_start(w3, wg_d[smp].rearrange("ck c m -> c ck m"))
                    ws.append(w)
                # PSUM -> SBUF with +EPS and cast to bf16
                nc.scalar.activation(yt0[:], tp0[:], AF.Copy, bias=EPS)
                nc.scalar.activation(yt1[:], tp1[:], AF.Copy, bias=EPS)
                gq = gpp.tile([128, 512], F32, tag="gq")
                for si in range(4):
                    sl = slice(si * 128, (si + 1) * 128)
                    nc.tensor.matmul(gq[:, sl], ws[si][:, 0:128], yt0[:, sl], start=True, stop=False)
                    nc.tensor.matmul(gq[:, sl], ws[si][:, 128:256], yt1[:, sl], start=False, stop=True)
                # one strided copy: [128m,(si,t)] -> G[128m, t, 4b] at b-offset 4q
                gq3 = gq[:].rearrange("p (si t) -> p si t", si=4)
                outv = g3[:, :, q * 4:q * 4 + 4].rearrange("p t b -> p b t")
                nc.scalar.activation(outv, gq3, AF.Copy, bias=0.0)

        def dp_step(t, pcur, pnxt):
            k, tl = divmod(t, TCH)
            yy = yyp.tile([128, 128], F32, tag="yy")
            nc.tensor.transpose(yy[:], gc3[k][:, tl, :], idf[:])
            a = apl.tile([128, 130], F32, tag="a")
            # A[s] = P[s] + e^-g * P[s-1]  for s=0..128 (col(s)=s+1; col0 = zero pad)
            nc.vector.scalar_tensor_tensor(a[:, 0:129], pcur[:, 0:129], E1,
                                           pcur[:, 1:130], OP.mult, OP.add)
            a3 = a[:].rearrange("p (s two) -> p s two", two=2)
            u3 = pnxt[:, 1:131].rearrange("p (s two) -> p s two", two=2)
            # even states: multiply by blank scalar (ScalarE, scale-AP)
            nc.scalar.activation(u3[:, :, 0], a3[:, :, 0], AF.Copy, bias=0.0,
                                 scale=ybe_sb[:, t:t + 1])
            # odd states: multiply by gathered label emissions
            nc.vector.tensor_tensor(u3[:, 0:64, 1], a3[:, 0:64, 1], yy[:, 0:64], OP.mult)
            # skip term
            v = vpl.tile([128, 64], F32, tag="v")
            p3 = pcur[:, 0:128].rearrange("p (s two) -> p s two", two=2)
            nc.vector.tensor_tensor(v[:], p3[:, :, 0], yy[:, 64:128], OP.mult)
            nc.vector.tensor_tensor(u3[:, 0:64, 1], u3[:, 0:64, 1], v[:], OP.add)
            if t % RBLK == RBLK - 1:
                ridx = t // RBLK
                mxc = mxh[:, ridx:ridx + 1]
                nc.vector.tensor_reduce(mxc, pnxt[:, 1:130], AX.X, OP.max)
                rec = spl.tile([128, 1], F32, tag="rec")
                nc.vector.reciprocal(rec[:], mxc)
                # scale so the row max sits at e^OFFS (keeps low states normal)
                rec2 = spl.tile([128, 1], F32, tag="rec2")
                nc.vector.tensor_scalar(rec2[:], rec[:], OFFE, None, OP.mult)
                nc.vector.tensor_scalar_mul(pnxt[:, 1:130], pnxt[:, 1:130], rec2[:])

        gather_chunk(0)
        gather_chunk(1)

        # init (t = 0): P[s=0] = ybe[:,0]; P~[s=1] = e^-g * y_lab(l=0,t=0)
        yy0 = yyp.tile([128, 128], F32, tag="yy")
        nc.tensor.transpose(yy0[:], gc3[0][:, 0, :], idf[:])
        nc.vector.tensor_copy(pa[:, 1:2], ybe_sb[:, 0:1])
        nc.vector.tensor_scalar(pa[:, 2:3], yy0[:, 0:1], E1, None, OP.mult)

        pcur, pnxt = pa, pb
        for t in range(1, T):
            k, tl = divmod(t, TCH)
            if tl == 1 and 2 <= k + 1 < NCH:
                gather_chunk(k + 1)
            dp_step(t, pcur, pnxt)
            pcur, pnxt = pnxt, pcur

        # final: export pend = sum(P * endmask) and the rescale history;
        # the exact logs happen on the host.
        scre = per.tile([128, S], F32, tag="scre", name="scre")
        nc.vector.tensor_tensor(scre[:], pcur[:, 1:130], em_sb[:], OP.mult)
        pend = per.tile([128, 1], F32, tag="pend", name="pend")
        nc.vector.tensor_reduce(pend[:], scre[:], AX.X, OP.add)
        nc.sync.dma_start(pend_d, pend[:])
        nc.sync.dma_start(mxh_d, mxh[:])

    nc.compile()
    return nc


def _host_derived(y_true, y_pred, label_length):
    import ml_dtypes

    lab = np.asarray(y_true, dtype=np.int64)  # [B, 64]
    llv = np.asarray(label_length).reshape(-1)
    # packed one-hots: [B, C, 128]; cols 0..63 labels (validity-masked),
    # cols 64..127 skip-masked labels scaled by e^(-2g)
    vm = (np.arange(L)[None, :] < llv[:, None])  # valid odd state s=2l+1
    zm = np.concatenate([np.zeros((B, 1), bool), lab[:, 1:] != lab[:, :-1]], axis=1)
    w = np.zeros((B, C, 128), dtype=np.float32)
    bb = np.repeat(np.arange(B), L)
    ll = np.tile(np.arange(L), B)
    cc = lab.reshape(-1)
    w[bb, cc, ll] = vm.reshape(-1).astype(np.float32)
    w[bb, cc, L + ll] = np.where(
        (zm & vm).reshape(-1),
        np.float32(np.exp(-2.0 * G_TILT)),
        w[bb, cc, L + ll],
    )
    wg = np.ascontiguousarray(
        w.reshape(B, 2, 128, 128).astype(ml_dtypes.bfloat16)
    )
    ybe = np.ascontiguousarray(np.asarray(y_pred)[:, :, C - 1] + np.float32(EPS))
    return wg, ybe


def kernel(y_true, y_pred, input_length, label_length, _trace=False):
    global _prog, _last_results
    from concourse.bass_utils import run_bass_kernel_spmd

    y_true = np.asarray(y_true)
    y_pred = np.asarray(y_pred, dtype=np.float32)
    label_length = np.asarray(label_length).reshape(-1)

    wg, ybe = _host_derived(y_true, y_pred, label_length)
    em = np.zeros((B, S), dtype=np.float32)
    bidx = np.arange(B)
    em[bidx, 2 * label_length] = 1.0
    em[bidx, 2 * label_length - 1] = np.float32(np.exp(-G_TILT))
    idf = np.eye(128, dtype=np.float32)

    if _prog is None:
        _prog = _build_program()

    in_maps = []
    for i in range(NCORES):
        sl = slice(i * BL, (i + 1) * BL)
        in_maps.append({
            "yp": np.ascontiguousarray(y_pred[sl]),
            "wg": wg[sl],
            "ybe": ybe[sl],
            "em": em[sl],
            "idf": idf,
        })
    res = run_bass_kernel_spmd(_prog, in_maps, core_ids=list(range(NCORES)),
                               trace=_trace)
    _last_results = res
    pend = np.concatenate([r["pend"] for r in res.results], axis=0).reshape(-1)
    mxh = np.concatenate([r["mxh"] for r in res.results], axis=0)
    nres = mxh.shape[1]
    logacc = np.log(mxh.astype(np.float64)).sum(axis=1) - OFFS * nres
    loss = -(np.log(pend.astype(np.float64)) + logacc
             + G_TILT * 2.0 * label_length.astype(np.float64))
    return loss.reshape(B, 1).astype(np.float32)


if __name__ == "__main__":
    rng = np.random.default_rng(0)
    yp = rng.random((B, T, C), dtype=np.float32)
    yp /= yp.sum(-1, keepdims=True)
    yt = rng.integers(0, C - 1, size=(B, L)).astype(np.int32)
    il = np.full((B, 1), T, dtype=np.int32)
    ll = rng.integers(32, L + 1, size=(B, 1)).astype(np.int32)
    print(kernel(yt, yp, il, ll)[:4])
